# revision 5
# baseline (speedup 1.0000x reference)
"""Trainium2 Bass kernel for nn_BinarizedRNN.

Math: the reference's output is out[t] = sign(hidden_t) @ sign(Wo).T where
hidden feeds the next step only through sign(hidden_t).  With l1,l2 > 0 the
SignSensitiveBatchNorm factor (s*l1 + (1-s)*l2)/sqrt(var+eps) is strictly
positive, so it never changes any sign; with sign(Wh) == I the recurrent
matmul is the identity.  The whole net collapses to

    q_t = (u'_t >= p_{t-1}),  p_t = q_t * (-2*g_{t+1}),   (elementwise)
    u'_t = x_t @ sign(Wi).T - g_t                         (one big matmul)
    out_t = (2*q_t - 1) @ sign(Wo).T

which maps to: one K=786-augmented matmul (hi/lo bf16 split, fp32-accurate),
a DVE tensor_tensor_scan(is_ge, mult) along time for the sign recurrence,
and an exact bf16 matmul for the output.  Data-parallel over B across 8
cores; no collectives needed (the batch-variance is provably inert).

Chain layout: rows are ordered (b, c) with c = 0 a reset column (u' = +BIG,
d1 = -g_1) so 4 independent b-chains of length 65 pack into one 260-column
r-tile and a single scan instruction handles all of them.
"""
import os
import numpy as np
import ml_dtypes

T, B, IN, H, OUT = 64, 256, 784, 2048, 256
EPS = 1e-5
NCORES = 8
BS = B // NCORES        # 32 batch rows per core
KAUG = IN + 2           # +g row, +reset row
CH = T + 1              # 65-column chains (reset + 64 steps)
NB = 4                  # b-chains per r-tile
RT = NB * CH            # 260
NRT = BS // NB          # 8 r-tiles per core
NHT = H // 128          # 16
NO = OUT // 128         # 2
RCOLS = BS * CH         # 2080 total row-columns per core
BIG = 1e9

# k-chunking of the 786-long contraction dim
KCHUNKS = []
_k0 = 0
while _k0 < KAUG:
    kn = min(128, KAUG - _k0)
    KCHUNKS.append((_k0, kn))
    _k0 += kn
KC = len(KCHUNKS)       # 7

KAUG2 = 2 * KAUG        # hilo2: hi rows stacked over lo rows
KCHUNKS2 = []
_k0 = 0
while _k0 < KAUG2:
    kn = min(128, KAUG2 - _k0)
    KCHUNKS2.append((_k0, kn))
    _k0 += kn

_CACHE = {}

# ---- dr8 mode constants: CH padded to 68 so fp8 pair-strides are 16B-aligned
CH8 = 68                 # reset + 64 steps + 3 zero-pad cols
RT8 = NB * CH8           # 272 cols per r-tile (psum tile)
RCOLS8 = BS * CH8        # 2176 cols per core
NRT8 = BS // NB          # 8 r-tiles
KLO = 768                # fp8-DR lo rows (3 chunks of 256)
KHI = IN + 2 + (IN - KLO)  # 802: x_hi + g + reset + 16 bf16 lo-tail rows
LO_SCALE = 64.0          # x_lo scaled by 64, w_lo = sign/64 (2^-6, e4m3-exact)
GRP8 = 4                 # r-tiles per weight load in the main matmul
NHP = NHT // 2           # 8 ht pairs for the fp8-DR output matmul
KCHUNKS8 = []
_k0 = 0
while _k0 < KHI:
    _kn = min(128, KHI - _k0)
    KCHUNKS8.append((_k0, _kn))
    _k0 += _kn


def _build_dr8(iters: int = 1):
    """bf16-hi + fp8-DoubleRow-lo main matmul, fp8-DR output matmul."""
    import contextlib
    import concourse.bacc as bacc
    import concourse.mybir as mybir
    import concourse.tile as tile

    f32 = mybir.dt.float32
    bf16 = mybir.dt.bfloat16
    f8 = mybir.dt.float8e4
    DR = mybir.MatmulPerfMode.DoubleRow

    nc = bacc.Bacc(
        "TRN2", target_bir_lowering=False, debug=False, num_devices=NCORES
    )
    xhi_d = nc.dram_tensor("xhi", [KHI, RCOLS8], bf16, kind="ExternalInput")
    xlo_d = nc.dram_tensor("xlo", [KLO, RCOLS8], f8, kind="ExternalInput")
    whi_d = nc.dram_tensor("whi", [KHI, H], bf16, kind="ExternalInput")
    wlo_d = nc.dram_tensor("wlo", [KLO, H], f8, kind="ExternalInput")
    wo_d = nc.dram_tensor("wo", [H, OUT], f8, kind="ExternalInput")
    d1_d = nc.dram_tensor("d1", [128, RT8], f32, kind="ExternalInput")
    outt_d = nc.dram_tensor("outt", [OUT, RCOLS8], f32, kind="ExternalOutput")

    assert iters == 1 or iters % 2 == 0
    n_phase = 1 if iters == 1 else 2

    with tile.TileContext(nc) as tc:
        with (
            tc.tile_pool(name="xp", bufs=1) as xp,
            tc.tile_pool(name="wp", bufs=1) as wp,
            tc.tile_pool(name="ppp", bufs=1) as ppp,
            tc.tile_pool(name="stage", bufs=4) as stage,
            tc.tile_pool(name="ps", bufs=8, space="PSUM") as ps,
            (tc.For_i(0, iters // 2, 1) if iters > 1 else contextlib.nullcontext()),
        ):
            for phase in range(n_phase):
                sfx = f"_{phase}"
                # ---- input DMAs (x ping-pongs across phases; w re-loads)
                xh_tiles = []
                wh_tiles = []
                for ci, (k0, kn) in enumerate(KCHUNKS8):
                    xt = xp.tile([kn, RCOLS8], bf16, tag=f"xh{ci}{sfx}",
                                 name=f"xh{ci}{sfx}")
                    nc.sync.dma_start(xt[:], xhi_d[k0 : k0 + kn, :])
                    xh_tiles.append(xt)
                    wt = wp.tile([kn, H], bf16, tag=f"wh{ci}", name=f"wh{ci}{sfx}")
                    nc.sync.dma_start(wt[:], whi_d[k0 : k0 + kn, :])
                    wh_tiles.append(wt)
                xl = xp.tile([128, KLO // 128, RCOLS8], f8, tag=f"xl{sfx}",
                             name=f"xl{sfx}")
                nc.sync.dma_start(xl[:], xlo_d.rearrange("(c p) n -> p c n", p=128))
                wl = wp.tile([128, KLO // 128, H], f8, tag="wl", name=f"wl{sfx}")
                nc.sync.dma_start(wl[:], wlo_d.rearrange("(c p) h -> p c h", p=128))
                wo8 = wp.tile([128, NHT, OUT], f8, tag="wo", name=f"wo{sfx}")
                nc.sync.dma_start(wo8[:], wo_d.rearrange("(c p) o -> p c o", p=128))
                d1t = wp.tile([128, RT8], f32, tag="d1", name=f"d1{sfx}")
                nc.sync.dma_start(d1t[:], d1_d[:])

                pp = [
                    ppp.tile([128, 2, RCOLS8], f8, tag=f"pp{hp}",
                             name=f"pp{hp}{sfx}")
                    for hp in range(NHP)
                ]

                # ---- main matmul + scan
                n_mm = len(KCHUNKS8) + KLO // 256
                for g in range(NRT8 // GRP8):
                    rts = list(range(g * GRP8, (g + 1) * GRP8))
                    for ht in range(NHT):
                        pss = [
                            ps.tile([128, RT8], f32, tag="ps",
                                    name=f"ps{sfx}_{g}_{ht}_{j}")
                            for j in range(GRP8)
                        ]
                        i = 0
                        for ci in range(len(KCHUNKS8)):
                            for j, rt in enumerate(rts):
                                nc.tensor.matmul(
                                    pss[j][:],
                                    wh_tiles[ci][:, ht * 128 : (ht + 1) * 128],
                                    xh_tiles[ci][:, rt * RT8 : (rt + 1) * RT8],
                                    start=(i == 0),
                                    stop=False,
                                )
                            i += 1
                        for c in range(KLO // 256):
                            for j, rt in enumerate(rts):
                                nc.tensor.matmul(
                                    pss[j][:],
                                    wl[:, 2 * c : 2 * c + 2,
                                       ht * 128 : (ht + 1) * 128],
                                    xl[:, 2 * c : 2 * c + 2,
                                       rt * RT8 : (rt + 1) * RT8],
                                    perf_mode=DR,
                                    start=False,
                                    stop=(c == KLO // 256 - 1),
                                )
                            i += 1
                        hp, a = ht // 2, ht % 2
                        for j, rt in enumerate(rts):
                            nc.vector.tensor_tensor_scan(
                                pp[hp][:, a : a + 1,
                                       rt * RT8 : (rt + 1) * RT8].rearrange(
                                    "p a b -> p (a b)"
                                ),
                                pss[j][:],
                                d1t[:],
                                0.0,
                                mybir.AluOpType.is_ge,
                                mybir.AluOpType.mult,
                            )

                # ---- output matmul: fp8-DR over ht pairs, weights reused
                # across all column blocks
                for o in range(NO):
                    pos = []
                    for hp in range(NHP):
                        for blk in range(NRT8):
                            if hp == 0:
                                pos.append(
                                    ps.tile([128, RT8], f32, tag="ps",
                                            name=f"po{sfx}_{o}_{blk}")
                                )
                            nc.tensor.matmul(
                                pos[blk][:],
                                wo8[:, 2 * hp : 2 * hp + 2,
                                    o * 128 : (o + 1) * 128],
                                pp[hp][:, :, blk * RT8 : (blk + 1) * RT8],
                                perf_mode=DR,
                                start=(hp == 0),
                                stop=(hp == NHP - 1),
                            )
                    for blk in range(NRT8):
                        st = stage.tile([128, RT8], f32, tag="st",
                                        name=f"st{sfx}_{o}_{blk}")
                        nc.vector.tensor_copy(st[:], pos[blk][:])
                        nc.sync.dma_start(
                            outt_d[o * 128 : (o + 1) * 128,
                                   blk * RT8 : (blk + 1) * RT8],
                            st[:],
                        )

    nc.compile()
    return nc


def _build(mode: str, iters: int = 1):
    """Build the SPMD Bacc module. mode in {"hilo", "fp32"}."""
    import concourse.bacc as bacc
    import concourse.mybir as mybir
    import concourse.tile as tile

    f32 = mybir.dt.float32
    f32r = mybir.dt.float32r
    bf16 = mybir.dt.bfloat16

    nc = bacc.Bacc(
        "TRN2", target_bir_lowering=False, debug=False, num_devices=NCORES
    )

    if mode == "hilo2":
        xs_d = nc.dram_tensor("xs", [KAUG2, RCOLS], bf16, kind="ExternalInput")
        wi_d = nc.dram_tensor("wi", [KAUG, H], bf16, kind="ExternalInput")
    elif mode == "hilo":
        xhi_d = nc.dram_tensor("xhi", [KAUG, RCOLS], bf16, kind="ExternalInput")
        xlo_d = nc.dram_tensor("xlo", [KAUG, RCOLS], bf16, kind="ExternalInput")
        wi_d = nc.dram_tensor("wi", [KAUG, H], bf16, kind="ExternalInput")
    else:
        xt_d = nc.dram_tensor("xt", [KAUG, RCOLS], f32, kind="ExternalInput")
        wi_d = nc.dram_tensor("wi", [KAUG, H], f32, kind="ExternalInput")
    sb_dt = {"hilo": bf16, "hilo2": bf16, "fp32": f32, "fp32r": f32r}[mode]
    wo_d = nc.dram_tensor("wo", [H, OUT], bf16, kind="ExternalInput")
    d1_d = nc.dram_tensor("d1", [128, RT], f32, kind="ExternalInput")
    outt_d = nc.dram_tensor("outt", [OUT, BS * T], f32, kind="ExternalOutput")


    with tile.TileContext(nc) as tc:
        import contextlib
        with (
            tc.tile_pool(name="xw", bufs=1) as xw,
            tc.tile_pool(name="ppool", bufs=20) as ppool,
            tc.tile_pool(name="stage", bufs=4) as stage,
            tc.tile_pool(name="ps1", bufs=6, space="PSUM") as ps1,
            tc.tile_pool(name="ps2", bufs=2, space="PSUM") as ps2,
            (tc.For_i(0, iters, 1) if iters > 1 else contextlib.nullcontext()),
        ):
            # resident inputs
            w_tiles = []
            x_tiles = []  # list of tuples (per pass)
            if mode == "hilo2":
                for ci, (k0, kn) in enumerate(KCHUNKS2):
                    wt = xw.tile([kn, H], bf16, tag=f"w{ci}")
                    # weight rows repeat with period KAUG (hi and lo share W)
                    a0 = k0 % KAUG
                    n1 = min(kn, KAUG - a0)
                    nc.sync.dma_start(wt[:n1, :], wi_d[a0 : a0 + n1, :])
                    if n1 < kn:
                        nc.sync.dma_start(wt[n1:kn, :], wi_d[0 : kn - n1, :])
                    w_tiles.append(wt)
                    xt_ = xw.tile([kn, RCOLS], bf16, tag=f"xs{ci}")
                    nc.sync.dma_start(xt_[:], xs_d[k0 : k0 + kn, :])
                    x_tiles.append((xt_,))
            for ci, (k0, kn) in enumerate(KCHUNKS if mode != "hilo2" else []):
                wt = xw.tile([kn, H], sb_dt, tag=f"w{ci}")
                if mode == "fp32r":
                    nc.gpsimd.dma_start(wt[:], wi_d[k0 : k0 + kn, :])
                else:
                    nc.sync.dma_start(wt[:], wi_d[k0 : k0 + kn, :])
                w_tiles.append(wt)
                if mode == "hilo":
                    xh = xw.tile([kn, RCOLS], bf16, tag=f"xh{ci}")
                    xl = xw.tile([kn, RCOLS], bf16, tag=f"xl{ci}")
                    nc.sync.dma_start(xh[:], xhi_d[k0 : k0 + kn, :])
                    nc.sync.dma_start(xl[:], xlo_d[k0 : k0 + kn, :])
                    x_tiles.append((xh, xl))
                elif mode == "fp32":
                    xf = xw.tile([kn, RCOLS], f32, tag=f"xf{ci}")
                    nc.sync.dma_start(xf[:], xt_d[k0 : k0 + kn, :])
                    x_tiles.append((xf,))
                else:
                    xf = xw.tile([kn, RCOLS], f32r, tag=f"xr{ci}")
                    nc.gpsimd.dma_start(xf[:], xt_d[k0 : k0 + kn, :])
                    x_tiles.append((xf,))
            wo_t = xw.tile([128, NHT, OUT], bf16, tag="wo")
            nc.sync.dma_start(wo_t[:], wo_d.rearrange("(c p) o -> p c o", p=128))
            d1_t = xw.tile([128, RT], f32, tag="d1")
            nc.sync.dma_start(d1_t[:], d1_d[:])

            n_pass = len(x_tiles[0])
            n_mm = len(w_tiles) * n_pass
            if os.environ.get("BASS_NN_STRUCT", "v1") == "v2":
                # v2: ht-pairs with k-outermost (PE consumes X chunks as DMA
                # delivers them -> no cold-start stall) + incremental output
                # matmul accumulation (no end tail).  GRP fixed at 2.
                GRP, HTP = 2, 2
                for g in range(NRT // GRP):
                    rts = list(range(g * GRP, (g + 1) * GRP))
                    p_tiles = []
                    po = {}
                    for hp in range(NHT // HTP):
                        pss = [
                            [
                                ps1.tile([128, RT], f32, tag="mm1",
                                         name=f"ps_{g}_{hp}_{a}_{j}")
                                for j in range(GRP)
                            ]
                            for a in range(HTP)
                        ]
                        for i, (ci, xp) in enumerate(
                            (ci, xp)
                            for ci in range(len(w_tiles))
                            for xp in range(n_pass)
                        ):
                            for a in range(HTP):
                                ht = hp * HTP + a
                                for j, rt in enumerate(rts):
                                    nc.tensor.matmul(
                                        pss[a][j][:],
                                        w_tiles[ci][:, ht * 128 : (ht + 1) * 128],
                                        x_tiles[ci][xp][:, rt * RT : (rt + 1) * RT],
                                        start=(i == 0),
                                        stop=(i == n_mm - 1),
                                    )
                        for a in range(HTP):
                            p = ppool.tile([128, GRP * NB, CH], bf16, tag="p",
                                           name=f"p_{g}_{hp}_{a}")
                            for j in range(GRP):
                                nc.vector.tensor_tensor_scan(
                                    p[:, j * NB : (j + 1) * NB, :].rearrange(
                                        "p a b -> p (a b)"
                                    ),
                                    pss[a][j][:],
                                    d1_t[:],
                                    0.0,
                                    mybir.AluOpType.is_ge,
                                    mybir.AluOpType.mult,
                                )
                            p_tiles.append(p)
                        # incremental output-matmul accumulation over ht
                        for o in range(NO):
                            if hp == 0:
                                po[o] = ps2.tile([128, GRP * NB * T], f32,
                                                 tag="mm2", name=f"po_{g}_{o}")
                            for a in range(HTP):
                                ht = hp * HTP + a
                                nc.tensor.matmul(
                                    po[o][:],
                                    wo_t[:, ht, o * 128 : (o + 1) * 128],
                                    p_tiles[ht][:, :, 1:],
                                    start=(ht == 0),
                                    stop=(ht == NHT - 1),
                                )
                    for o in range(NO):
                        st = stage.tile([128, GRP * NB * T], f32, tag="st",
                                        name=f"st_{g}_{o}")
                        nc.vector.tensor_copy(st[:], po[o][:])
                        col = g * GRP * NB * T
                        nc.sync.dma_start(
                            outt_d[o * 128 : (o + 1) * 128, col : col + GRP * NB * T],
                            st[:],
                        )
            else:
                GRP = int(os.environ.get("BASS_NN_GRP", "2"))  # r-tiles per group
                n_mm = KC * n_pass
                for g in range(NRT // GRP):
                    rts = list(range(g * GRP, (g + 1) * GRP))
                    p_tiles = []              # one [128, GRP*NB, CH] tile per ht
                    for ht in range(NHT):
                        pss = [ps1.tile([128, RT], f32, tag="mm1", name=f"ps_{g}_{ht}_{j}") for j in range(len(rts))]
                        for i, (ci, xp) in enumerate(
                            (ci, xp)
                            for ci in range(len(w_tiles))
                            for xp in range(n_pass)
                        ):
                            for j, rt in enumerate(rts):
                                nc.tensor.matmul(
                                    pss[j][:],
                                    w_tiles[ci][:, ht * 128 : (ht + 1) * 128],
                                    x_tiles[ci][xp][:, rt * RT : (rt + 1) * RT],
                                    start=(i == 0),
                                    stop=(i == n_mm - 1),
                                )
                        p = ppool.tile([128, GRP * NB, CH], bf16, tag="p")
                        ablate = os.environ.get("BASS_NN_ABLATE", "none")
                        for j in range(GRP):
                            pv = p[:, j * NB : (j + 1) * NB, :].rearrange(
                                "p a b -> p (a b)"
                            )
                            if ablate == "noscan":
                                nc.vector.tensor_copy(pv, pss[j][:])
                            else:
                                nc.vector.tensor_tensor_scan(
                                    pv,
                                    pss[j][:],
                                    d1_t[:],
                                    0.0,
                                    mybir.AluOpType.is_ge,
                                    mybir.AluOpType.mult,
                                )
                        p_tiles.append(p)
                    # output matmuls: rt-pairs -> N=512, skip reset columns
                    PW = 2 if GRP % 2 == 0 else 1
                    for pr in range(0 if os.environ.get("BASS_NN_ABLATE") == "nomm2" else GRP // PW):
                        for o in range(NO):
                            po = ps2.tile([128, PW * NB * T], f32, tag="mm2")
                            for ht in range(NHT):
                                nc.tensor.matmul(
                                    po[:],
                                    wo_t[:, ht, o * 128 : (o + 1) * 128],
                                    p_tiles[ht][:, PW * NB * pr : PW * NB * (pr + 1), 1:],
                                    start=(ht == 0),
                                    stop=(ht == NHT - 1),
                                )
                            st = stage.tile([128, PW * NB * T], f32, tag="st")
                            nc.vector.tensor_copy(st[:], po[:])
                            col = (g * GRP + PW * pr) * NB * T
                            nc.sync.dma_start(
                                outt_d[o * 128 : (o + 1) * 128, col : col + PW * NB * T],
                                st[:],
                            )

    nc.compile()
    return nc


def _get_module(mode, iters=1):
    key = (mode, iters, os.environ.get("BASS_NN_GRP", "2"),
           os.environ.get("BASS_NN_ABLATE", "none"),
           os.environ.get("BASS_NN_STRUCT", "v1"))
    if key not in _CACHE:
        _CACHE[key] = _build_dr8(iters) if mode == "dr8" else _build(mode, iters)
    return _CACHE[key]


def _fallback_numpy(x, Wi, Wh, Wo, gates, l1, l2):
    """Direct fp32 replication of the reference for degenerate inputs."""
    Wi_b = np.sign(Wi)
    Wh_b = np.sign(Wh)
    Wo_b = np.sign(Wo)
    Bn, Hn = x.shape[1], Wi.shape[0]
    h = np.zeros((Bn, Hn), dtype=np.float32)
    outs = []
    for t in range(x.shape[0]):
        hidden = x[t] @ Wi_b.T + gates[t] * (np.sign(h) @ Wh_b.T)
        hidden = np.clip(hidden, -1.0, 1.0)
        var = hidden.var(axis=0, ddof=1, keepdims=True)
        bottom = np.sqrt(var + EPS)
        s = 1.0 / (1.0 + np.exp(-10.0 * hidden))
        hidden = (hidden * s * l1 + hidden * (1.0 - s) * l2) / bottom
        outs.append(np.sign(hidden) @ Wo_b.T)
        h = hidden
    return np.stack(outs).astype(np.float32)


def _dd_vec(gates, ch):
    """Per-chain scan multiplier: dd[0]=-g_0, dd[1+t]=-2*gamma_t, rest 0."""
    gamma = np.empty(T, dtype=np.float32)
    gamma[: T - 1] = gates[1:]
    gamma[T - 1] = 1.0
    dd = np.zeros(ch, dtype=np.float32)
    dd[0] = -gates[0]
    dd[1 : 1 + T] = -2.0 * gamma
    return dd, gamma


def _prep_dr8(x, gates, Wi_b, Wo_b):
    """Build per-core input maps for dr8 mode."""
    f8 = ml_dtypes.float8_e4m3
    whi = np.empty((KHI, H), dtype=np.float32)
    whi[:IN] = Wi_b.T
    whi[IN] = -1.0
    whi[IN + 1] = 1.0
    whi[IN + 2 :] = Wi_b.T[KLO:IN]
    whi = whi.astype(ml_dtypes.bfloat16)
    wlo = (Wi_b.T[:KLO] / LO_SCALE).astype(f8)
    wo8 = np.ascontiguousarray(Wo_b.T).astype(f8)
    dd, gamma = _dd_vec(gates, CH8)
    d1 = np.tile(np.tile(dd, NB)[None, :], (128, 1)).astype(np.float32)
    g_bf = gates.astype(ml_dtypes.bfloat16).astype(np.float32)

    in_maps = []
    for c in range(NCORES):
        xs = x[:, c * BS : (c + 1) * BS, :]               # [T, BS, IN]
        xs_t = np.ascontiguousarray(xs.transpose(2, 1, 0))  # [IN, BS, T]
        hi = xs_t.astype(ml_dtypes.bfloat16)
        lo = xs_t - hi.astype(np.float32)                  # [IN, BS, T]
        xa = np.zeros((KHI, BS, CH8), dtype=ml_dtypes.bfloat16)
        xa[:IN, :, 1 : 1 + T] = hi
        xa[IN, :, 1 : 1 + T] = g_bf[None, :]
        xa[IN + 1, :, 0] = BIG
        xa[IN + 2 :, :, 1 : 1 + T] = lo[KLO:IN].astype(ml_dtypes.bfloat16)
        xl = np.zeros((KLO, BS, CH8), dtype=f8)
        xl[:, :, 1 : 1 + T] = (lo[:KLO] * LO_SCALE).astype(f8)
        in_maps.append({
            "xhi": xa.reshape(KHI, RCOLS8),
            "xlo": xl.reshape(KLO, RCOLS8),
            "whi": whi,
            "wlo": wlo,
            "wo": wo8,
            "d1": d1,
        })
    return in_maps, gamma


def _prep_in_maps(x, gates, wi_aug, wo_arr, d1, mode):
    """Per-core X^T with augmentation rows and reset columns: [KAUG, BS*CH].
    Column order: (b, c) with c=0 reset, c>=1 -> timestep c-1."""
    in_maps = []
    if mode == "hilo":
        wi_hi = wi_aug.astype(ml_dtypes.bfloat16)
        wi_lo = (wi_aug - wi_hi.astype(np.float32)).astype(ml_dtypes.bfloat16)
        # weights are +-1/0 and small aug values: hi is exact, lo == 0
        assert np.all(wi_lo.astype(np.float32) == 0.0)
    for c in range(NCORES):
        xs = x[:, c * BS : (c + 1) * BS, :]             # [T, BS, IN]
        xa = np.zeros((KAUG, BS, CH), dtype=np.float32)
        xa[:IN, :, 1:] = xs.transpose(2, 1, 0)          # [IN, BS, T]
        xa[IN, :, 1:] = gates[None, :]                  # g_t row
        xa[IN + 1, :, 0] = BIG                          # reset row
        xa = xa.reshape(KAUG, RCOLS)
        m = {"wo": wo_arr, "d1": d1}
        if mode == "hilo2":
            xhi = xa.astype(ml_dtypes.bfloat16)
            xlo = (xa - xhi.astype(np.float32)).astype(ml_dtypes.bfloat16)
            m["xs"] = np.vstack([xhi, xlo])
            m["wi"] = wi_aug.astype(ml_dtypes.bfloat16)
        elif mode == "hilo":
            xhi = xa.astype(ml_dtypes.bfloat16)
            xlo = (xa - xhi.astype(np.float32)).astype(ml_dtypes.bfloat16)
            m["xhi"] = xhi
            m["xlo"] = xlo
            m["wi"] = wi_hi
        else:
            m["xt"] = xa
            m["wi"] = wi_aug
        in_maps.append(m)
    return in_maps


LAST_RESULTS = None


def kernel(x, Wi, Wh, Wo, gates, l1, l2):
    global LAST_RESULTS
    x = np.asarray(x, dtype=np.float32)
    Wi = np.asarray(Wi, dtype=np.float32)
    Wh = np.asarray(Wh, dtype=np.float32)
    Wo = np.asarray(Wo, dtype=np.float32)
    gates = np.asarray(gates, dtype=np.float32)
    l1 = np.asarray(l1, dtype=np.float32)
    l2 = np.asarray(l2, dtype=np.float32)

    fast = (
        x.shape == (T, B, IN)
        and np.all(l1 > 0)
        and np.all(l2 > 0)
        and np.array_equal(np.sign(Wh), np.eye(H, dtype=np.float32))
        and np.all(gates[1:] != 0)
    )
    if not fast:
        return _fallback_numpy(x, Wi, Wh, Wo, gates, l1, l2)

    from concourse.bass_utils import run_bass_kernel_spmd

    mode = os.environ.get("BASS_NN_MODE", "dr8")
    if mode == "dr8":
        # dr8 stores the scan output in fp8 and the g-row in bf16: require
        # the -2g/-g multipliers and gates to be exactly representable.
        dd, _ = _dd_vec(gates, CH8)
        f8 = ml_dtypes.float8_e4m3
        if not (
            np.array_equal(dd.astype(f8).astype(np.float32), dd)
            and np.array_equal(
                gates.astype(ml_dtypes.bfloat16).astype(np.float32), gates
            )
        ):
            mode = "hilo"
    nc = _get_module(mode)

    Wi_b = np.sign(Wi)                      # [H, IN]
    Wo_b = np.sign(Wo)                      # [OUT, H]
    colsum = Wo_b.sum(axis=1)               # [OUT]

    if mode == "dr8":
        in_maps, gamma = _prep_dr8(x, gates, Wi_b, Wo_b)
        res = run_bass_kernel_spmd(nc, in_maps, core_ids=list(range(NCORES)))
        LAST_RESULTS = res
        out = np.empty((T, B, OUT), dtype=np.float32)
        inv_gamma = (1.0 / gamma).astype(np.float32)
        for c in range(NCORES):
            ot = res.results[c]["outt"].reshape(OUT, BS, CH8)
            out[:, c * BS : (c + 1) * BS, :] = (
                -ot[:, :, 1 : 1 + T].transpose(2, 1, 0)
                * inv_gamma[:, None, None]
                - colsum[None, None, :]
            )
        return out

    # augmented, transposed input-weights: [KAUG, H]
    wi_aug = np.empty((KAUG, H), dtype=np.float32)
    wi_aug[:IN] = Wi_b.T
    wi_aug[IN] = -1.0                       # g row
    wi_aug[IN + 1] = 1.0                    # reset row
    wo_arr = np.ascontiguousarray(Wo_b.T).astype(ml_dtypes.bfloat16)  # [H, OUT]

    # d1 per chain column: c=0 -> -g_1 ; c=1..63 -> -2*g_{c+1} ; c=64 -> -2
    gamma = np.empty(T, dtype=np.float32)   # scale for output recovery
    gamma[: T - 1] = gates[1:]
    gamma[T - 1] = 1.0
    dd = np.empty(CH, dtype=np.float32)
    dd[0] = -gates[0]
    dd[1:] = -2.0 * gamma
    d1 = np.tile(np.tile(dd, NB)[None, :], (128, 1)).astype(np.float32)

    in_maps = _prep_in_maps(x, gates, wi_aug, wo_arr, d1, mode)
    res = run_bass_kernel_spmd(nc, in_maps, core_ids=list(range(NCORES)))
    LAST_RESULTS = res

    out = np.empty((T, B, OUT), dtype=np.float32)
    inv_gamma = (1.0 / gamma).astype(np.float32)        # [T]
    for c in range(NCORES):
        ot = res.results[c]["outt"].reshape(OUT, BS, T)
        # out[t, b, o] = -ot[o, b, t]/gamma[t] - colsum[o]
        out[:, c * BS : (c + 1) * BS, :] = (
            -ot.transpose(2, 1, 0) * inv_gamma[:, None, None]
            - colsum[None, None, :]
        )
    return out



# revision 14
# speedup vs baseline: 1.4942x; 1.4942x over previous
"""Trainium2 Bass kernel for nn_BinarizedRNN.

Math: the reference's output is out[t] = sign(hidden_t) @ sign(Wo).T where
hidden feeds the next step only through sign(hidden_t).  With l1,l2 > 0 the
SignSensitiveBatchNorm factor (s*l1 + (1-s)*l2)/sqrt(var+eps) is strictly
positive, so it never changes any sign; with sign(Wh) == I the recurrent
matmul is the identity.  The whole net collapses to

    q_t = (u'_t >= p_{t-1}),  p_t = q_t * (-2*g_{t+1}),   (elementwise)
    u'_t = x_t @ sign(Wi).T - g_t                         (one big matmul)
    out_t = (2*q_t - 1) @ sign(Wo).T

which maps to: one K=786-augmented matmul (hi/lo bf16 split, fp32-accurate),
a DVE tensor_tensor_scan(is_ge, mult) along time for the sign recurrence,
and an exact bf16 matmul for the output.  Data-parallel over B across 8
cores; no collectives needed (the batch-variance is provably inert).

Chain layout: rows are ordered (b, c) with c = 0 a reset column (u' = +BIG,
d1 = -g_1) so 4 independent b-chains of length 65 pack into one 260-column
r-tile and a single scan instruction handles all of them.
"""
import os
import numpy as np
import ml_dtypes

T, B, IN, H, OUT = 64, 256, 784, 2048, 256
EPS = 1e-5
NCORES = 8
BS = B // NCORES        # 32 batch rows per core
KAUG = IN + 2           # +g row, +reset row
CH = T + 1              # 65-column chains (reset + 64 steps)
NB = 4                  # b-chains per r-tile
RT = NB * CH            # 260
NRT = BS // NB          # 8 r-tiles per core
NHT = H // 128          # 16
NO = OUT // 128         # 2
RCOLS = BS * CH         # 2080 total row-columns per core
BIG = 1e9

# k-chunking of the 786-long contraction dim
KCHUNKS = []
_k0 = 0
while _k0 < KAUG:
    kn = min(128, KAUG - _k0)
    KCHUNKS.append((_k0, kn))
    _k0 += kn
KC = len(KCHUNKS)       # 7

KAUG2 = 2 * KAUG        # hilo2: hi rows stacked over lo rows
KCHUNKS2 = []
_k0 = 0
while _k0 < KAUG2:
    kn = min(128, KAUG2 - _k0)
    KCHUNKS2.append((_k0, kn))
    _k0 += kn

_CACHE = {}

# ---- f83 main-matmul variant: all-fp8 3-level (hi + lo*16 + lolo*256)
KQ = 2560                # 10 DR chunks of 256 rows (2354 used + zero pad)
NQC = KQ // 256          # 10
S1, S2 = 16.0, 256.0     # residual scales (weights 2^-4, 2^-8 — fp8-exact)

# ---- dr8 mode constants: CH padded to 68 so fp8 pair-strides are 16B-aligned
CH8 = 68                 # reset + 64 steps + 3 zero-pad cols
RT8 = NB * CH8           # 272 cols per r-tile (psum tile)
RCOLS8 = BS * CH8        # 2176 cols per core
NRT8 = BS // NB          # 8 r-tiles
KLO = 768                # fp8-DR lo rows (3 chunks of 256)
KHI = IN + 2 + (IN - KLO)  # 802: x_hi + g + reset + 16 bf16 lo-tail rows
LO_SCALE = 64.0          # x_lo scaled by 64, w_lo = sign/64 (2^-6, e4m3-exact)
GRP8 = 4                 # r-tiles per weight load in the main matmul
NHP = NHT // 2           # 8 ht pairs for the fp8-DR output matmul
KCHUNKS8 = []
_k0 = 0
while _k0 < KHI:
    _kn = min(128, KHI - _k0)
    KCHUNKS8.append((_k0, _kn))
    _k0 += _kn


def _build_dr8(iters: int = 1):
    """bf16-hi + fp8-DoubleRow-lo main matmul, fp8-DR output matmul."""
    import contextlib
    import concourse.bacc as bacc
    import concourse.mybir as mybir
    import concourse.tile as tile

    f32 = mybir.dt.float32
    bf16 = mybir.dt.bfloat16
    f8 = mybir.dt.float8e4
    DR = mybir.MatmulPerfMode.DoubleRow

    nc = bacc.Bacc(
        "TRN2", target_bir_lowering=False, debug=False, num_devices=NCORES
    )
    xhi_d = nc.dram_tensor("xhi", [KHI, RCOLS8], bf16, kind="ExternalInput")
    xlo_d = nc.dram_tensor("xlo", [KLO, RCOLS8], f8, kind="ExternalInput")
    whi_d = nc.dram_tensor("whi", [KHI, H], bf16, kind="ExternalInput")
    wlo_d = nc.dram_tensor("wlo", [KLO, H], f8, kind="ExternalInput")
    wo_d = nc.dram_tensor("wo", [H, OUT], f8, kind="ExternalInput")
    d1_d = nc.dram_tensor("d1", [128, RT8], f32, kind="ExternalInput")
    outt_d = nc.dram_tensor("outt", [OUT, RCOLS8], f32, kind="ExternalOutput")

    assert iters == 1 or iters % 2 == 0
    n_phase = 1 if iters == 1 else 2

    with tile.TileContext(nc) as tc:
        with (
            tc.tile_pool(name="xp", bufs=1) as xp,
            tc.tile_pool(name="wp", bufs=1) as wp,
            tc.tile_pool(name="ppp", bufs=1) as ppp,
            tc.tile_pool(name="stage", bufs=4) as stage,
            tc.tile_pool(name="ps", bufs=8, space="PSUM") as ps,
            (tc.For_i(0, iters // 2, 1) if iters > 1 else contextlib.nullcontext()),
        ):
            for phase in range(n_phase):
                sfx = f"_{phase}"
                # ---- input DMAs (x ping-pongs across phases; w re-loads)
                xh_tiles = []
                wh_tiles = []
                for ci, (k0, kn) in enumerate(KCHUNKS8):
                    xt = xp.tile([kn, RCOLS8], bf16, tag=f"xh{ci}{sfx}",
                                 name=f"xh{ci}{sfx}")
                    nc.sync.dma_start(xt[:], xhi_d[k0 : k0 + kn, :])
                    xh_tiles.append(xt)
                    wt = wp.tile([kn, H], bf16, tag=f"wh{ci}", name=f"wh{ci}{sfx}")
                    nc.sync.dma_start(wt[:], whi_d[k0 : k0 + kn, :])
                    wh_tiles.append(wt)
                xl = xp.tile([128, KLO // 128, RCOLS8], f8, tag=f"xl{sfx}",
                             name=f"xl{sfx}")
                nc.sync.dma_start(xl[:], xlo_d.rearrange("(c p) n -> p c n", p=128))
                wl = wp.tile([128, KLO // 128, H], f8, tag="wl", name=f"wl{sfx}")
                nc.sync.dma_start(wl[:], wlo_d.rearrange("(c p) h -> p c h", p=128))
                wo8 = wp.tile([128, NHT, OUT], f8, tag="wo", name=f"wo{sfx}")
                nc.sync.dma_start(wo8[:], wo_d.rearrange("(c p) o -> p c o", p=128))
                d1t = wp.tile([128, RT8], f32, tag="d1", name=f"d1{sfx}")
                nc.sync.dma_start(d1t[:], d1_d[:])

                pp = [
                    ppp.tile([128, 2, RCOLS8], f8, tag=f"pp{hp}",
                             name=f"pp{hp}{sfx}")
                    for hp in range(NHP)
                ]

                # ---- main matmul + scan
                n_mm = len(KCHUNKS8) + KLO // 256
                for g in range(NRT8 // GRP8):
                    rts = list(range(g * GRP8, (g + 1) * GRP8))
                    for ht in range(NHT):
                        pss = [
                            ps.tile([128, RT8], f32, tag="ps",
                                    name=f"ps{sfx}_{g}_{ht}_{j}")
                            for j in range(GRP8)
                        ]
                        i = 0
                        for ci in range(len(KCHUNKS8)):
                            for j, rt in enumerate(rts):
                                nc.tensor.matmul(
                                    pss[j][:],
                                    wh_tiles[ci][:, ht * 128 : (ht + 1) * 128],
                                    xh_tiles[ci][:, rt * RT8 : (rt + 1) * RT8],
                                    start=(i == 0),
                                    stop=False,
                                )
                            i += 1
                        for c in range(KLO // 256):
                            for j, rt in enumerate(rts):
                                nc.tensor.matmul(
                                    pss[j][:],
                                    wl[:, 2 * c : 2 * c + 2,
                                       ht * 128 : (ht + 1) * 128],
                                    xl[:, 2 * c : 2 * c + 2,
                                       rt * RT8 : (rt + 1) * RT8],
                                    perf_mode=DR,
                                    start=False,
                                    stop=(c == KLO // 256 - 1),
                                )
                            i += 1
                        hp, a = ht // 2, ht % 2
                        for j, rt in enumerate(rts):
                            nc.vector.tensor_tensor_scan(
                                pp[hp][:, a : a + 1,
                                       rt * RT8 : (rt + 1) * RT8].rearrange(
                                    "p a b -> p (a b)"
                                ),
                                pss[j][:],
                                d1t[:],
                                0.0,
                                mybir.AluOpType.is_ge,
                                mybir.AluOpType.mult,
                            )

                # ---- output matmul: fp8-DR over ht pairs, weights reused
                # across all column blocks
                for o in range(NO):
                    pos = []
                    for hp in range(NHP):
                        for blk in range(NRT8):
                            if hp == 0:
                                pos.append(
                                    ps.tile([128, RT8], f32, tag="ps",
                                            name=f"po{sfx}_{o}_{blk}")
                                )
                            nc.tensor.matmul(
                                pos[blk][:],
                                wo8[:, 2 * hp : 2 * hp + 2,
                                    o * 128 : (o + 1) * 128],
                                pp[hp][:, :, blk * RT8 : (blk + 1) * RT8],
                                perf_mode=DR,
                                start=(hp == 0),
                                stop=(hp == NHP - 1),
                            )
                    for blk in range(NRT8):
                        st = stage.tile([128, RT8], f32, tag="st",
                                        name=f"st{sfx}_{o}_{blk}")
                        nc.vector.tensor_copy(st[:], pos[blk][:])
                        nc.sync.dma_start(
                            outt_d[o * 128 : (o + 1) * 128,
                                   blk * RT8 : (blk + 1) * RT8],
                            st[:],
                        )

    nc.compile()
    return nc


def _build_d2(iters: int = 1, main: str = "mix"):
    """v3: main matmul (CH=65) as bf16-hi + fp8-DR-lo ("mix") or all-fp8
    3-level DoubleRow ("f83"); psum->bf16 scans on DVE, bf16 output matmul
    (weights reused across 8 col blocks), stage copies on the Act engine."""
    import contextlib
    import concourse.bacc as bacc
    import concourse.mybir as mybir
    import concourse.tile as tile

    f32 = mybir.dt.float32
    bf16 = mybir.dt.bfloat16
    f8 = mybir.dt.float8e4
    DR = mybir.MatmulPerfMode.DoubleRow

    nc = bacc.Bacc(
        "TRN2", target_bir_lowering=False, debug=False, num_devices=NCORES
    )
    if main == "f83":
        xq_d = nc.dram_tensor("xq", [KQ, RCOLS], f8, kind="ExternalInput")
        wq_d = nc.dram_tensor("wq", [KQ, H], f8, kind="ExternalInput")
    else:
        xhi_d = nc.dram_tensor("xhi", [KHI, RCOLS], bf16, kind="ExternalInput")
        xlo_d = nc.dram_tensor("xlo", [KLO, RCOLS], f8, kind="ExternalInput")
        whi_d = nc.dram_tensor("whi", [KHI, H], bf16, kind="ExternalInput")
        wlo_d = nc.dram_tensor("wlo", [KLO, H], f8, kind="ExternalInput")
    wo_d = nc.dram_tensor("wo", [H, OUT], bf16, kind="ExternalInput")
    d1_d = nc.dram_tensor("d1", [128, RT], f32, kind="ExternalInput")
    outt_d = nc.dram_tensor("outt", [OUT, RCOLS], f32, kind="ExternalOutput")

    with tile.TileContext(nc) as tc:
        with (
            tc.tile_pool(name="xp", bufs=1) as xp,
            tc.tile_pool(name="wp", bufs=1) as wp,
            tc.tile_pool(name="ppp", bufs=1) as ppp,
            tc.tile_pool(name="stage", bufs=4) as stage,
            tc.tile_pool(name="ps", bufs=8, space="PSUM") as ps,
            (tc.For_i(0, iters, 1) if iters > 1 else contextlib.nullcontext()),
        ):
            if main == "f83":
                xq = xp.tile([128, KQ // 128, RCOLS], f8, tag="xq", name="xq")
                nc.sync.dma_start(xq[:], xq_d.rearrange("(c p) n -> p c n", p=128))
                wq = wp.tile([128, KQ // 128, H], f8, tag="wq", name="wq")
                nc.sync.dma_start(wq[:], wq_d.rearrange("(c p) h -> p c h", p=128))
            else:
                xh_tiles = []
                wh_tiles = []
                for ci, (k0, kn) in enumerate(KCHUNKS8):
                    xt = xp.tile([kn, RCOLS], bf16, tag=f"xh{ci}", name=f"xh{ci}")
                    nc.sync.dma_start(xt[:], xhi_d[k0 : k0 + kn, :])
                    xh_tiles.append(xt)
                    wt = wp.tile([kn, H], bf16, tag=f"wh{ci}", name=f"wh{ci}")
                    nc.sync.dma_start(wt[:], whi_d[k0 : k0 + kn, :])
                    wh_tiles.append(wt)
                xl = xp.tile([128, KLO // 128, RCOLS], f8, tag="xl", name="xl")
                nc.sync.dma_start(xl[:], xlo_d.rearrange("(c p) n -> p c n", p=128))
                wl = wp.tile([128, KLO // 128, H], f8, tag="wl", name="wl")
                nc.sync.dma_start(wl[:], wlo_d.rearrange("(c p) h -> p c h", p=128))
            wo_t = wp.tile([128, NHT, OUT], bf16, tag="wo", name="wo")
            nc.sync.dma_start(wo_t[:], wo_d.rearrange("(c p) o -> p c o", p=128))
            d1t = wp.tile([128, RT], f32, tag="d1", name="d1")
            nc.sync.dma_start(d1t[:], d1_d[:])

            pp = [
                ppp.tile([128, RCOLS], bf16, tag=f"pp{ht}", name=f"pp{ht}")
                for ht in range(NHT)
            ]

            # ---- main matmul + scans
            for g in range(NRT // GRP8):
                rts = list(range(g * GRP8, (g + 1) * GRP8))
                for ht in range(NHT):
                    pss = [
                        ps.tile([128, RT], f32, tag="ps", name=f"ps_{g}_{ht}_{j}")
                        for j in range(GRP8)
                    ]
                    if main == "f83":
                        for c in range(NQC):
                            for j, rt in enumerate(rts):
                                nc.tensor.matmul(
                                    pss[j][:],
                                    wq[:, 2 * c : 2 * c + 2,
                                       ht * 128 : (ht + 1) * 128],
                                    xq[:, 2 * c : 2 * c + 2,
                                       rt * RT : (rt + 1) * RT],
                                    perf_mode=DR,
                                    start=(c == 0),
                                    stop=(c == NQC - 1),
                                )
                    else:
                        for ci in range(len(KCHUNKS8)):
                            for j, rt in enumerate(rts):
                                nc.tensor.matmul(
                                    pss[j][:],
                                    wh_tiles[ci][:, ht * 128 : (ht + 1) * 128],
                                    xh_tiles[ci][:, rt * RT : (rt + 1) * RT],
                                    start=(ci == 0),
                                    stop=False,
                                )
                        for c in range(KLO // 256):
                            for j, rt in enumerate(rts):
                                nc.tensor.matmul(
                                    pss[j][:],
                                    wl[:, 2 * c : 2 * c + 2,
                                       ht * 128 : (ht + 1) * 128],
                                    xl[:, 2 * c : 2 * c + 2,
                                       rt * RT : (rt + 1) * RT],
                                    perf_mode=DR,
                                    start=False,
                                    stop=(c == KLO // 256 - 1),
                                )
                    for j, rt in enumerate(rts):
                        nc.vector.tensor_tensor_scan(
                            pp[ht][:, rt * RT : (rt + 1) * RT],
                            pss[j][:],
                            d1t[:],
                            0.0,
                            mybir.AluOpType.is_ge,
                            mybir.AluOpType.mult,
                        )

            # ---- bf16 output matmul, weights reused across 8 col blocks
            for o in range(NO):
                pos = []
                for ht in range(NHT):
                    for blk in range(NRT):
                        if ht == 0:
                            pos.append(
                                ps.tile([128, RT], f32, tag="ps",
                                        name=f"po_{o}_{blk}")
                            )
                        nc.tensor.matmul(
                            pos[blk][:],
                            wo_t[:, ht, o * 128 : (o + 1) * 128],
                            pp[ht][:, blk * RT : (blk + 1) * RT],
                            start=(ht == 0),
                            stop=(ht == NHT - 1),
                        )
                for blk in range(NRT):
                    st = stage.tile([128, RT], f32, tag="st",
                                    name=f"st_{o}_{blk}")
                    nc.scalar.activation(
                        st[:], pos[blk][:], mybir.ActivationFunctionType.Copy
                    )
                    nc.sync.dma_start(
                        outt_d[o * 128 : (o + 1) * 128,
                               blk * RT : (blk + 1) * RT],
                        st[:],
                    )

    nc.compile()
    return nc


def _build(mode: str, iters: int = 1):
    """Build the SPMD Bacc module. mode in {"hilo", "fp32"}."""
    import concourse.bacc as bacc
    import concourse.mybir as mybir
    import concourse.tile as tile

    f32 = mybir.dt.float32
    f32r = mybir.dt.float32r
    bf16 = mybir.dt.bfloat16

    nc = bacc.Bacc(
        "TRN2", target_bir_lowering=False, debug=False, num_devices=NCORES
    )

    if mode == "hilo2":
        xs_d = nc.dram_tensor("xs", [KAUG2, RCOLS], bf16, kind="ExternalInput")
        wi_d = nc.dram_tensor("wi", [KAUG, H], bf16, kind="ExternalInput")
    elif mode == "hilo":
        xhi_d = nc.dram_tensor("xhi", [KAUG, RCOLS], bf16, kind="ExternalInput")
        xlo_d = nc.dram_tensor("xlo", [KAUG, RCOLS], bf16, kind="ExternalInput")
        wi_d = nc.dram_tensor("wi", [KAUG, H], bf16, kind="ExternalInput")
    else:
        xt_d = nc.dram_tensor("xt", [KAUG, RCOLS], f32, kind="ExternalInput")
        wi_d = nc.dram_tensor("wi", [KAUG, H], f32, kind="ExternalInput")
    sb_dt = {"hilo": bf16, "hilo2": bf16, "fp32": f32, "fp32r": f32r}[mode]
    wo_d = nc.dram_tensor("wo", [H, OUT], bf16, kind="ExternalInput")
    d1_d = nc.dram_tensor("d1", [128, RT], f32, kind="ExternalInput")
    outt_d = nc.dram_tensor("outt", [OUT, BS * T], f32, kind="ExternalOutput")


    with tile.TileContext(nc) as tc:
        import contextlib
        with (
            tc.tile_pool(name="xw", bufs=1) as xw,
            tc.tile_pool(name="ppool", bufs=20) as ppool,
            tc.tile_pool(name="stage", bufs=4) as stage,
            tc.tile_pool(name="ps1", bufs=6, space="PSUM") as ps1,
            tc.tile_pool(name="ps2", bufs=2, space="PSUM") as ps2,
            (tc.For_i(0, iters, 1) if iters > 1 else contextlib.nullcontext()),
        ):
            # resident inputs
            w_tiles = []
            x_tiles = []  # list of tuples (per pass)
            if mode == "hilo2":
                for ci, (k0, kn) in enumerate(KCHUNKS2):
                    wt = xw.tile([kn, H], bf16, tag=f"w{ci}")
                    # weight rows repeat with period KAUG (hi and lo share W)
                    a0 = k0 % KAUG
                    n1 = min(kn, KAUG - a0)
                    nc.sync.dma_start(wt[:n1, :], wi_d[a0 : a0 + n1, :])
                    if n1 < kn:
                        nc.sync.dma_start(wt[n1:kn, :], wi_d[0 : kn - n1, :])
                    w_tiles.append(wt)
                    xt_ = xw.tile([kn, RCOLS], bf16, tag=f"xs{ci}")
                    nc.sync.dma_start(xt_[:], xs_d[k0 : k0 + kn, :])
                    x_tiles.append((xt_,))
            for ci, (k0, kn) in enumerate(KCHUNKS if mode != "hilo2" else []):
                wt = xw.tile([kn, H], sb_dt, tag=f"w{ci}")
                if mode == "fp32r":
                    nc.gpsimd.dma_start(wt[:], wi_d[k0 : k0 + kn, :])
                else:
                    nc.sync.dma_start(wt[:], wi_d[k0 : k0 + kn, :])
                w_tiles.append(wt)
                if mode == "hilo":
                    xh = xw.tile([kn, RCOLS], bf16, tag=f"xh{ci}")
                    xl = xw.tile([kn, RCOLS], bf16, tag=f"xl{ci}")
                    nc.sync.dma_start(xh[:], xhi_d[k0 : k0 + kn, :])
                    nc.sync.dma_start(xl[:], xlo_d[k0 : k0 + kn, :])
                    x_tiles.append((xh, xl))
                elif mode == "fp32":
                    xf = xw.tile([kn, RCOLS], f32, tag=f"xf{ci}")
                    nc.sync.dma_start(xf[:], xt_d[k0 : k0 + kn, :])
                    x_tiles.append((xf,))
                else:
                    xf = xw.tile([kn, RCOLS], f32r, tag=f"xr{ci}")
                    nc.gpsimd.dma_start(xf[:], xt_d[k0 : k0 + kn, :])
                    x_tiles.append((xf,))
            wo_t = xw.tile([128, NHT, OUT], bf16, tag="wo")
            nc.sync.dma_start(wo_t[:], wo_d.rearrange("(c p) o -> p c o", p=128))
            d1_t = xw.tile([128, RT], f32, tag="d1")
            nc.sync.dma_start(d1_t[:], d1_d[:])

            n_pass = len(x_tiles[0])
            n_mm = len(w_tiles) * n_pass
            if os.environ.get("BASS_NN_STRUCT", "v1") == "v2":
                # v2: ht-pairs with k-outermost (PE consumes X chunks as DMA
                # delivers them -> no cold-start stall) + incremental output
                # matmul accumulation (no end tail).  GRP fixed at 2.
                GRP, HTP = 2, 2
                for g in range(NRT // GRP):
                    rts = list(range(g * GRP, (g + 1) * GRP))
                    p_tiles = []
                    po = {}
                    for hp in range(NHT // HTP):
                        pss = [
                            [
                                ps1.tile([128, RT], f32, tag="mm1",
                                         name=f"ps_{g}_{hp}_{a}_{j}")
                                for j in range(GRP)
                            ]
                            for a in range(HTP)
                        ]
                        for i, (ci, xp) in enumerate(
                            (ci, xp)
                            for ci in range(len(w_tiles))
                            for xp in range(n_pass)
                        ):
                            for a in range(HTP):
                                ht = hp * HTP + a
                                for j, rt in enumerate(rts):
                                    nc.tensor.matmul(
                                        pss[a][j][:],
                                        w_tiles[ci][:, ht * 128 : (ht + 1) * 128],
                                        x_tiles[ci][xp][:, rt * RT : (rt + 1) * RT],
                                        start=(i == 0),
                                        stop=(i == n_mm - 1),
                                    )
                        for a in range(HTP):
                            p = ppool.tile([128, GRP * NB, CH], bf16, tag="p",
                                           name=f"p_{g}_{hp}_{a}")
                            for j in range(GRP):
                                nc.vector.tensor_tensor_scan(
                                    p[:, j * NB : (j + 1) * NB, :].rearrange(
                                        "p a b -> p (a b)"
                                    ),
                                    pss[a][j][:],
                                    d1_t[:],
                                    0.0,
                                    mybir.AluOpType.is_ge,
                                    mybir.AluOpType.mult,
                                )
                            p_tiles.append(p)
                        # incremental output-matmul accumulation over ht
                        for o in range(NO):
                            if hp == 0:
                                po[o] = ps2.tile([128, GRP * NB * T], f32,
                                                 tag="mm2", name=f"po_{g}_{o}")
                            for a in range(HTP):
                                ht = hp * HTP + a
                                nc.tensor.matmul(
                                    po[o][:],
                                    wo_t[:, ht, o * 128 : (o + 1) * 128],
                                    p_tiles[ht][:, :, 1:],
                                    start=(ht == 0),
                                    stop=(ht == NHT - 1),
                                )
                    for o in range(NO):
                        st = stage.tile([128, GRP * NB * T], f32, tag="st",
                                        name=f"st_{g}_{o}")
                        nc.vector.tensor_copy(st[:], po[o][:])
                        col = g * GRP * NB * T
                        nc.sync.dma_start(
                            outt_d[o * 128 : (o + 1) * 128, col : col + GRP * NB * T],
                            st[:],
                        )
            else:
                GRP = int(os.environ.get("BASS_NN_GRP", "2"))  # r-tiles per group
                n_mm = KC * n_pass
                for g in range(NRT // GRP):
                    rts = list(range(g * GRP, (g + 1) * GRP))
                    p_tiles = []              # one [128, GRP*NB, CH] tile per ht
                    for ht in range(NHT):
                        pss = [ps1.tile([128, RT], f32, tag="mm1", name=f"ps_{g}_{ht}_{j}") for j in range(len(rts))]
                        for i, (ci, xp) in enumerate(
                            (ci, xp)
                            for ci in range(len(w_tiles))
                            for xp in range(n_pass)
                        ):
                            for j, rt in enumerate(rts):
                                nc.tensor.matmul(
                                    pss[j][:],
                                    w_tiles[ci][:, ht * 128 : (ht + 1) * 128],
                                    x_tiles[ci][xp][:, rt * RT : (rt + 1) * RT],
                                    start=(i == 0),
                                    stop=(i == n_mm - 1),
                                )
                        p = ppool.tile([128, GRP * NB, CH], bf16, tag="p")
                        ablate = os.environ.get("BASS_NN_ABLATE", "none")
                        for j in range(GRP):
                            pv = p[:, j * NB : (j + 1) * NB, :].rearrange(
                                "p a b -> p (a b)"
                            )
                            if ablate == "noscan":
                                nc.vector.tensor_copy(pv, pss[j][:])
                            else:
                                nc.vector.tensor_tensor_scan(
                                    pv,
                                    pss[j][:],
                                    d1_t[:],
                                    0.0,
                                    mybir.AluOpType.is_ge,
                                    mybir.AluOpType.mult,
                                )
                        p_tiles.append(p)
                    # output matmuls: rt-pairs -> N=512, skip reset columns
                    PW = 2 if GRP % 2 == 0 else 1
                    for pr in range(0 if os.environ.get("BASS_NN_ABLATE") == "nomm2" else GRP // PW):
                        for o in range(NO):
                            po = ps2.tile([128, PW * NB * T], f32, tag="mm2")
                            for ht in range(NHT):
                                nc.tensor.matmul(
                                    po[:],
                                    wo_t[:, ht, o * 128 : (o + 1) * 128],
                                    p_tiles[ht][:, PW * NB * pr : PW * NB * (pr + 1), 1:],
                                    start=(ht == 0),
                                    stop=(ht == NHT - 1),
                                )
                            st = stage.tile([128, PW * NB * T], f32, tag="st")
                            nc.vector.tensor_copy(st[:], po[:])
                            col = (g * GRP + PW * pr) * NB * T
                            nc.sync.dma_start(
                                outt_d[o * 128 : (o + 1) * 128, col : col + PW * NB * T],
                                st[:],
                            )

    nc.compile()
    return nc


def _get_module(mode, iters=1):
    key = (mode, iters, os.environ.get("BASS_NN_GRP", "2"),
           os.environ.get("BASS_NN_ABLATE", "none"),
           os.environ.get("BASS_NN_STRUCT", "v1"),
           os.environ.get("BASS_NN_MAIN", "mix"))
    if key not in _CACHE:
        if mode == "d2":
            _CACHE[key] = _build_d2(iters, os.environ.get("BASS_NN_MAIN", "mix"))
        elif mode == "dr8":
            _CACHE[key] = _build_dr8(iters)
        else:
            _CACHE[key] = _build(mode, iters)
    return _CACHE[key]


def _fallback_numpy(x, Wi, Wh, Wo, gates, l1, l2):
    """Direct fp32 replication of the reference for degenerate inputs."""
    Wi_b = np.sign(Wi)
    Wh_b = np.sign(Wh)
    Wo_b = np.sign(Wo)
    Bn, Hn = x.shape[1], Wi.shape[0]
    h = np.zeros((Bn, Hn), dtype=np.float32)
    outs = []
    for t in range(x.shape[0]):
        hidden = x[t] @ Wi_b.T + gates[t] * (np.sign(h) @ Wh_b.T)
        hidden = np.clip(hidden, -1.0, 1.0)
        var = hidden.var(axis=0, ddof=1, keepdims=True)
        bottom = np.sqrt(var + EPS)
        s = 1.0 / (1.0 + np.exp(-10.0 * hidden))
        hidden = (hidden * s * l1 + hidden * (1.0 - s) * l2) / bottom
        outs.append(np.sign(hidden) @ Wo_b.T)
        h = hidden
    return np.stack(outs).astype(np.float32)


def _dd_vec(gates, ch):
    """Per-chain scan multiplier: dd[0]=-g_0, dd[1+t]=-2*gamma_t, rest 0."""
    gamma = np.empty(T, dtype=np.float32)
    gamma[: T - 1] = gates[1:]
    gamma[T - 1] = 1.0
    dd = np.zeros(ch, dtype=np.float32)
    dd[0] = -gates[0]
    dd[1 : 1 + T] = -2.0 * gamma
    return dd, gamma


def _prep_d2(x, gates, Wi_b, Wo_b, main: str = "mix"):
    """Per-core input maps for d2 mode (CH=65, bf16 wo, bf16 scan output)."""
    f8 = ml_dtypes.float8_e4m3
    wo_arr = np.ascontiguousarray(Wo_b.T).astype(ml_dtypes.bfloat16)
    dd, gamma = _dd_vec(gates, CH)
    d1 = np.tile(np.tile(dd, NB)[None, :], (128, 1)).astype(np.float32)
    g_bf = gates.astype(ml_dtypes.bfloat16).astype(np.float32)

    if main == "f83":
        wq = np.zeros((KQ, H), dtype=np.float32)
        wq[:IN] = Wi_b.T
        wq[IN : 2 * IN] = Wi_b.T / S1
        wq[2 * IN : 3 * IN] = Wi_b.T / S2
        wq[3 * IN] = -1.0          # g row
        wq[3 * IN + 1] = 1.0       # reset row
        wq = wq.astype(f8)
    else:
        whi = np.empty((KHI, H), dtype=np.float32)
        whi[:IN] = Wi_b.T
        whi[IN] = -1.0
        whi[IN + 1] = 1.0
        whi[IN + 2 :] = Wi_b.T[KLO:IN]
        whi = whi.astype(ml_dtypes.bfloat16)
        wlo = (Wi_b.T[:KLO] / LO_SCALE).astype(f8)

    in_maps = []
    for c in range(NCORES):
        xs = x[:, c * BS : (c + 1) * BS, :]               # [T, BS, IN]
        xs_t = np.ascontiguousarray(xs.transpose(2, 1, 0))  # [IN, BS, T]
        if main == "f83":
            hi8 = xs_t.astype(f8)
            r1 = xs_t - hi8.astype(np.float32)
            q1 = (r1 * S1).astype(f8)
            r2 = r1 - q1.astype(np.float32) / S1
            q2 = (r2 * S2).astype(f8)
            xqa = np.zeros((KQ, BS, CH), dtype=f8)
            xqa[:IN, :, 1:] = hi8
            xqa[IN : 2 * IN, :, 1:] = q1
            xqa[2 * IN : 3 * IN, :, 1:] = q2
            xqa[3 * IN, :, 1:] = g_bf[None, :]
            xqa[3 * IN + 1, :, 0] = 240.0
            in_maps.append({
                "xq": xqa.reshape(KQ, RCOLS),
                "wq": wq,
                "wo": wo_arr,
                "d1": d1,
            })
        else:
            hi = xs_t.astype(ml_dtypes.bfloat16)
            lo = xs_t - hi.astype(np.float32)              # [IN, BS, T]
            xa = np.zeros((KHI, BS, CH), dtype=ml_dtypes.bfloat16)
            xa[:IN, :, 1:] = hi
            xa[IN, :, 1:] = g_bf[None, :]
            xa[IN + 1, :, 0] = BIG
            xa[IN + 2 :, :, 1:] = lo[KLO:IN].astype(ml_dtypes.bfloat16)
            xl = np.zeros((KLO, BS, CH), dtype=f8)
            xl[:, :, 1:] = (lo[:KLO] * LO_SCALE).astype(f8)
            in_maps.append({
                "xhi": xa.reshape(KHI, RCOLS),
                "xlo": xl.reshape(KLO, RCOLS),
                "whi": whi,
                "wlo": wlo,
                "wo": wo_arr,
                "d1": d1,
            })
    return in_maps, gamma


def _prep_dr8(x, gates, Wi_b, Wo_b):
    """Build per-core input maps for dr8 mode."""
    f8 = ml_dtypes.float8_e4m3
    whi = np.empty((KHI, H), dtype=np.float32)
    whi[:IN] = Wi_b.T
    whi[IN] = -1.0
    whi[IN + 1] = 1.0
    whi[IN + 2 :] = Wi_b.T[KLO:IN]
    whi = whi.astype(ml_dtypes.bfloat16)
    wlo = (Wi_b.T[:KLO] / LO_SCALE).astype(f8)
    wo8 = np.ascontiguousarray(Wo_b.T).astype(f8)
    dd, gamma = _dd_vec(gates, CH8)
    d1 = np.tile(np.tile(dd, NB)[None, :], (128, 1)).astype(np.float32)
    g_bf = gates.astype(ml_dtypes.bfloat16).astype(np.float32)

    in_maps = []
    for c in range(NCORES):
        xs = x[:, c * BS : (c + 1) * BS, :]               # [T, BS, IN]
        xs_t = np.ascontiguousarray(xs.transpose(2, 1, 0))  # [IN, BS, T]
        hi = xs_t.astype(ml_dtypes.bfloat16)
        lo = xs_t - hi.astype(np.float32)                  # [IN, BS, T]
        xa = np.zeros((KHI, BS, CH8), dtype=ml_dtypes.bfloat16)
        xa[:IN, :, 1 : 1 + T] = hi
        xa[IN, :, 1 : 1 + T] = g_bf[None, :]
        xa[IN + 1, :, 0] = BIG
        xa[IN + 2 :, :, 1 : 1 + T] = lo[KLO:IN].astype(ml_dtypes.bfloat16)
        xl = np.zeros((KLO, BS, CH8), dtype=f8)
        xl[:, :, 1 : 1 + T] = (lo[:KLO] * LO_SCALE).astype(f8)
        in_maps.append({
            "xhi": xa.reshape(KHI, RCOLS8),
            "xlo": xl.reshape(KLO, RCOLS8),
            "whi": whi,
            "wlo": wlo,
            "wo": wo8,
            "d1": d1,
        })
    return in_maps, gamma


def _prep_in_maps(x, gates, wi_aug, wo_arr, d1, mode):
    """Per-core X^T with augmentation rows and reset columns: [KAUG, BS*CH].
    Column order: (b, c) with c=0 reset, c>=1 -> timestep c-1."""
    in_maps = []
    if mode == "hilo":
        wi_hi = wi_aug.astype(ml_dtypes.bfloat16)
        wi_lo = (wi_aug - wi_hi.astype(np.float32)).astype(ml_dtypes.bfloat16)
        # weights are +-1/0 and small aug values: hi is exact, lo == 0
        assert np.all(wi_lo.astype(np.float32) == 0.0)
    for c in range(NCORES):
        xs = x[:, c * BS : (c + 1) * BS, :]             # [T, BS, IN]
        xa = np.zeros((KAUG, BS, CH), dtype=np.float32)
        xa[:IN, :, 1:] = xs.transpose(2, 1, 0)          # [IN, BS, T]
        xa[IN, :, 1:] = gates[None, :]                  # g_t row
        xa[IN + 1, :, 0] = BIG                          # reset row
        xa = xa.reshape(KAUG, RCOLS)
        m = {"wo": wo_arr, "d1": d1}
        if mode == "hilo2":
            xhi = xa.astype(ml_dtypes.bfloat16)
            xlo = (xa - xhi.astype(np.float32)).astype(ml_dtypes.bfloat16)
            m["xs"] = np.vstack([xhi, xlo])
            m["wi"] = wi_aug.astype(ml_dtypes.bfloat16)
        elif mode == "hilo":
            xhi = xa.astype(ml_dtypes.bfloat16)
            xlo = (xa - xhi.astype(np.float32)).astype(ml_dtypes.bfloat16)
            m["xhi"] = xhi
            m["xlo"] = xlo
            m["wi"] = wi_hi
        else:
            m["xt"] = xa
            m["wi"] = wi_aug
        in_maps.append(m)
    return in_maps


LAST_RESULTS = None


def kernel(x, Wi, Wh, Wo, gates, l1, l2):
    global LAST_RESULTS
    x = np.asarray(x, dtype=np.float32)
    Wi = np.asarray(Wi, dtype=np.float32)
    Wh = np.asarray(Wh, dtype=np.float32)
    Wo = np.asarray(Wo, dtype=np.float32)
    gates = np.asarray(gates, dtype=np.float32)
    l1 = np.asarray(l1, dtype=np.float32)
    l2 = np.asarray(l2, dtype=np.float32)

    fast = (
        x.shape == (T, B, IN)
        and np.all(l1 > 0)
        and np.all(l2 > 0)
        and np.array_equal(np.sign(Wh), np.eye(H, dtype=np.float32))
        and np.all(gates[1:] != 0)
    )
    if not fast:
        return _fallback_numpy(x, Wi, Wh, Wo, gates, l1, l2)

    from concourse.bass_utils import run_bass_kernel_spmd

    mode = os.environ.get("BASS_NN_MODE", "d2")
    if mode in ("d2", "dr8"):
        # these modes store the scan output in bf16/fp8 and the g-row in
        # bf16: require the -2g/-g multipliers and gates exactly representable.
        ch = CH8 if mode == "dr8" else CH
        dt_chk = ml_dtypes.float8_e4m3 if mode == "dr8" else ml_dtypes.bfloat16
        dd, _ = _dd_vec(gates, ch)
        if not (
            np.array_equal(dd.astype(dt_chk).astype(np.float32), dd)
            and np.array_equal(
                gates.astype(ml_dtypes.bfloat16).astype(np.float32), gates
            )
        ):
            mode = "hilo"
    nc = _get_module(mode)

    Wi_b = np.sign(Wi)                      # [H, IN]
    Wo_b = np.sign(Wo)                      # [OUT, H]
    colsum = Wo_b.sum(axis=1)               # [OUT]

    if mode in ("d2", "dr8"):
        ch = CH if mode == "d2" else CH8
        if mode == "d2":
            in_maps, gamma = _prep_d2(
                x, gates, Wi_b, Wo_b, os.environ.get("BASS_NN_MAIN", "mix")
            )
        else:
            in_maps, gamma = _prep_dr8(x, gates, Wi_b, Wo_b)
        res = run_bass_kernel_spmd(nc, in_maps, core_ids=list(range(NCORES)))
        LAST_RESULTS = res
        out = np.empty((T, B, OUT), dtype=np.float32)
        inv_gamma = (1.0 / gamma).astype(np.float32)
        for c in range(NCORES):
            ot = res.results[c]["outt"].reshape(OUT, BS, ch)
            out[:, c * BS : (c + 1) * BS, :] = (
                -ot[:, :, 1 : 1 + T].transpose(2, 1, 0)
                * inv_gamma[:, None, None]
                - colsum[None, None, :]
            )
        return out

    # augmented, transposed input-weights: [KAUG, H]
    wi_aug = np.empty((KAUG, H), dtype=np.float32)
    wi_aug[:IN] = Wi_b.T
    wi_aug[IN] = -1.0                       # g row
    wi_aug[IN + 1] = 1.0                    # reset row
    wo_arr = np.ascontiguousarray(Wo_b.T).astype(ml_dtypes.bfloat16)  # [H, OUT]

    # d1 per chain column: c=0 -> -g_1 ; c=1..63 -> -2*g_{c+1} ; c=64 -> -2
    gamma = np.empty(T, dtype=np.float32)   # scale for output recovery
    gamma[: T - 1] = gates[1:]
    gamma[T - 1] = 1.0
    dd = np.empty(CH, dtype=np.float32)
    dd[0] = -gates[0]
    dd[1:] = -2.0 * gamma
    d1 = np.tile(np.tile(dd, NB)[None, :], (128, 1)).astype(np.float32)

    in_maps = _prep_in_maps(x, gates, wi_aug, wo_arr, d1, mode)
    res = run_bass_kernel_spmd(nc, in_maps, core_ids=list(range(NCORES)))
    LAST_RESULTS = res

    out = np.empty((T, B, OUT), dtype=np.float32)
    inv_gamma = (1.0 / gamma).astype(np.float32)        # [T]
    for c in range(NCORES):
        ot = res.results[c]["outt"].reshape(OUT, BS, T)
        # out[t, b, o] = -ot[o, b, t]/gamma[t] - colsum[o]
        out[:, c * BS : (c + 1) * BS, :] = (
            -ot.transpose(2, 1, 0) * inv_gamma[:, None, None]
            - colsum[None, None, :]
        )
    return out

